# revision 1
# baseline (speedup 1.0000x reference)
"""DipNetEncoder Trainium2 kernel: 8-way batch-parallel, bf16 compute.

Per core (B_loc=256): activations resident in SBUF.
- X_L1 [128(o_lo), 512(b*2+o_hi), 96(n_pad)] bf16 : feature-matmul input layout
- feature mm per node (K=i on partitions) -> Y PSUM -> Y1 [128, 2(oh), BH, 128(n_pad)]
- xbar transpose quarters -> Y2 [128(n), 32(b), 2(oh), 128(o_lo)] -> A-mix (A^T stationary)
- Z chunks: ACT copy (+sum) + DVE square (+sumsq), spilled to DRAM bf16
- BN stats AllReduce across 8 cores; s,t per node; affine+relu on Z chunks,
  xbar back to L1, fused residual add in place.
"""
import os
import numpy as np
import ml_dtypes

B = 2048
N = 81
NP = 96
BO_IN, PO_IN = 35, 40
EMB = 256
NUM_BLOCKS = int(os.environ.get("KERNEL_NUM_BLOCKS", "8"))
BN_EPS = 1e-5
N_CORES = 8
BL = B // N_CORES   # 256
BQ = 32             # b-quarter for Y2/A-mix staging
BC = 4              # b-chunk for post-stats affine pipeline
NELEM_GLOBAL = float(B * EMB)

_cache = {}


def _build():
    import concourse.bacc as bacc
    import concourse.mybir as mybir
    import concourse.tile as tile

    BF = mybir.dt.bfloat16
    F32 = mybir.dt.float32
    AF = mybir.ActivationFunctionType
    ALU = mybir.AluOpType

    nc = bacc.Bacc("TRN2", target_bir_lowering=False, debug=False, num_devices=N_CORES)

    d_in = {}
    for half, cin in (("bo", BO_IN), ("po", PO_IN)):
        d_in[f"x_{half}"] = nc.dram_tensor(f"x_{half}", [cin, N, BL], BF, kind="ExternalInput").ap()
        d_in[f"w0_{half}"] = nc.dram_tensor(f"w0_{half}", [cin, N, EMB], BF, kind="ExternalInput").ap()
        if NUM_BLOCKS > 1:
            d_in[f"w_{half}"] = nc.dram_tensor(
                f"w_{half}", [NUM_BLOCKS - 1, 128, 2, N, EMB], BF, kind="ExternalInput").ap()
        d_in[f"g_{half}"] = nc.dram_tensor(f"g_{half}", [NP, 2 * NUM_BLOCKS], F32, kind="ExternalInput").ap()
    d_in["amat"] = nc.dram_tensor("amat", [N, N], BF, kind="ExternalInput").ap()
    d_out = {
        h: nc.dram_tensor(f"out_{h}", [128, 2 * BL, NP], BF, kind="ExternalOutput").ap()
        for h in ("bo", "po")
    }

    with tile.TileContext(nc) as tc:
        with (
            tc.tile_pool(name="persist", bufs=1) as pp,
            tc.tile_pool(name="dram", bufs=1, space="DRAM") as dp,
        ):
            a_t = pp.tile([N, N], BF)
            nc.sync.dma_start(out=a_t[:], in_=d_in["amat"])
            z_dram = dp.tile([N, BL * EMB], BF)
            cc_in = dp.tile([NP, 2], F32)
            cc_out = dp.tile([NP, 2], F32)

            for half in ("bo", "po"):
                cin = BO_IN if half == "bo" else PO_IN
                with tc.tile_pool(name=f"xh_{half}", bufs=1) as xp:
                    x1 = xp.tile([128, 2 * BL, NP], BF)
                    x1v = x1[:].rearrange("p (b h) n -> p b h n", h=2)
                    gb = xp.tile([NP, 2 * NUM_BLOCKS], F32)
                    nc.sync.dma_start(out=gb[:], in_=d_in[f"g_{half}"])

                    for blk in range(NUM_BLOCKS):
                        BH = 64 if blk == 0 else 128   # sub-batch size for Y staging
                        nsub = BL // BH
                        with (
                            tc.tile_pool(name=f"mm_{half}_{blk}", bufs=1) as yp,
                            tc.tile_pool(name=f"w_{half}_{blk}", bufs=2) as wp,
                            tc.tile_pool(name=f"c_{half}_{blk}", bufs=2) as cp,
                            tc.tile_pool(name=f"st_{half}_{blk}", bufs=1) as sp,
                            tc.tile_pool(name=f"psA_{half}_{blk}", bufs=2, space="PSUM") as psA,
                            tc.tile_pool(name=f"psB_{half}_{blk}", bufs=2, space="PSUM") as psB,
                        ):
                            s1_parts = sp.tile([N, BL * EMB // 512], F32)
                            s2_parts = sp.tile([N, BL * EMB // 512], F32)
                            zi = 0
                            for sb in range(nsub):
                                if blk == 0:
                                    x0s = yp.tile([cin, N, BH], BF, tag="x0")
                                    nc.sync.dma_start(
                                        out=x0s[:],
                                        in_=d_in[f"x_{half}"][:, :, sb * BH:(sb + 1) * BH])
                                y1 = yp.tile([128, 2, BH, 128], BF, tag="y1")
                                for g0 in range(0, N, 4):
                                    gn = min(4, N - g0)
                                    if blk > 0:
                                        wg = wp.tile([128, 2, 4, EMB], BF, tag="w")
                                        nc.sync.dma_start(
                                            out=wg[:, :, :gn, :],
                                            in_=d_in[f"w_{half}"][blk - 1][:, :, g0:g0 + gn, :])
                                    else:
                                        wg0 = wp.tile([cin, 4, EMB], BF, tag="w0")
                                        nc.sync.dma_start(
                                            out=wg0[:, :gn, :],
                                            in_=d_in[f"w0_{half}"][:, g0:g0 + gn, :])
                                    for oh in range(2):
                                        ps = psA.tile([128, 4, BH], F32, tag="psA")
                                        for j in range(gn):
                                            n_ = g0 + j
                                            if blk == 0:
                                                nc.tensor.matmul(
                                                    ps[:, j],
                                                    wg0[:, j, oh * 128:(oh + 1) * 128],
                                                    x0s[:, n_, :], start=True, stop=True)
                                            else:
                                                for ih in range(2):
                                                    rhs = x1v[:, sb * BH:(sb + 1) * BH, ih, n_]
                                                    nc.tensor.matmul(
                                                        ps[:, j],
                                                        wg[:, ih, j, oh * 128:(oh + 1) * 128],
                                                        rhs, start=(ih == 0), stop=(ih == 1))
                                        dst = y1[:, oh, :, g0:g0 + gn].rearrange("p b n -> p n b")
                                        if (g0 // 4) % 2 == 0:
                                            nc.scalar.activation(dst, ps[:, :gn], AF.Copy)
                                        else:
                                            nc.vector.tensor_copy(dst, ps[:, :gn])
                                for q in range(BH // BQ):
                                    y2q = yp.tile([128, BQ, 2, 128], BF, tag="y2")
                                    for oh in range(2):
                                        nc.sync.dma_start(
                                            out=y2q[:, :, oh, :],
                                            in_=y1[:, oh, q * BQ:(q + 1) * BQ, :],
                                            transpose=True)
                                    y2f = y2q[:N].rearrange("n b h o -> n (b h o)")
                                    for c in range(BQ * EMB // 512):
                                        zp = psB.tile([N, 512], F32, tag="psB")
                                        nc.tensor.matmul(zp[:], a_t[:], y2f[:, c * 512:(c + 1) * 512],
                                                         start=True, stop=True)
                                        z_sb = cp.tile([N, 512], BF, tag="zsb")
                                        nc.scalar.activation(z_sb[:], zp[:], AF.Copy,
                                                             accum_out=s1_parts[:, zi:zi + 1])
                                        sq = cp.tile([N, 512], BF, tag="sq")
                                        nc.vector.scalar_tensor_tensor(
                                            sq[:], z_sb[:], 1.0, z_sb[:], ALU.mult, ALU.mult,
                                            accum_out=s2_parts[:, zi:zi + 1])
                                        nc.sync.dma_start(
                                            out=z_dram[:, zi * 512:(zi + 1) * 512], in_=z_sb[:])
                                        zi += 1

                            # ---- stats -> AllReduce -> s,t ----
                            st = sp.tile([NP, 2], F32)
                            nc.vector.memset(st[:], 0.0)
                            nc.vector.tensor_reduce(st[:N, 0:1], s1_parts[:], mybir.AxisListType.X, ALU.add)
                            nc.vector.tensor_reduce(st[:N, 1:2], s2_parts[:], mybir.AxisListType.X, ALU.add)
                            nc.gpsimd.dma_start(cc_in[:], st[:])
                            nc.gpsimd.collective_compute(
                                "AllReduce", ALU.add,
                                replica_groups=[list(range(N_CORES))],
                                ins=[cc_in.opt()], outs=[cc_out.opt()])
                            glob = sp.tile([NP, 2], F32)
                            nc.gpsimd.dma_start(glob[:], cc_out[:])
                            mu = sp.tile([NP, 1], F32)
                            va = sp.tile([NP, 1], F32)
                            s_f = sp.tile([NP, 1], F32)
                            t_f = sp.tile([NP, 1], F32)
                            rt = sp.tile([NP, 1], F32)
                            nc.vector.tensor_scalar_mul(mu[:], glob[:, 0:1], 1.0 / NELEM_GLOBAL)
                            nc.vector.tensor_scalar_mul(va[:], glob[:, 1:2], 1.0 / NELEM_GLOBAL)
                            msq = sp.tile([NP, 1], F32)
                            nc.vector.scalar_tensor_tensor(msq[:], mu[:], 0.0, mu[:], ALU.bypass, ALU.mult)
                            nc.vector.scalar_tensor_tensor(va[:], va[:], 0.0, msq[:], ALU.bypass, ALU.subtract)
                            nc.vector.tensor_scalar_add(va[:], va[:], BN_EPS)
                            nc.scalar.activation(rt[:], va[:], AF.Sqrt)
                            nc.vector.reciprocal(s_f[:], rt[:])
                            nc.vector.scalar_tensor_tensor(
                                s_f[:], s_f[:], 0.0, gb[:, 2 * blk:2 * blk + 1], ALU.bypass, ALU.mult)
                            nc.vector.scalar_tensor_tensor(
                                t_f[:], mu[:], -1.0, s_f[:], ALU.mult, ALU.mult)
                            nc.vector.scalar_tensor_tensor(
                                t_f[:], t_f[:], 0.0, gb[:, 2 * blk + 1:2 * blk + 2], ALU.bypass, ALU.add)

                            # ---- affine+relu(+residual) chunks, xbar back to L1 ----
                            for c in range(BL // BC):
                                zc = cp.tile([NP, BC * EMB], BF, tag="zc")
                                nc.sync.dma_start(
                                    out=zc[:N], in_=z_dram[:, c * BC * EMB:(c + 1) * BC * EMB])
                                uc = cp.tile([NP, BC * EMB], BF, tag="uc")
                                nc.vector.tensor_scalar(uc[:N], zc[:N], s_f[:N], t_f[:N],
                                                        ALU.mult, ALU.add)
                                ul = cp.tile([128, BC * 2, NP], BF, tag="ul")
                                nc.sync.dma_start(out=ul[:], in_=uc[:], transpose=True)
                                xs = x1[:, c * BC * 2:(c + 1) * BC * 2, :]
                                if blk == 0:
                                    nc.vector.tensor_scalar_max(xs, ul[:], 0.0)
                                else:
                                    nc.vector.scalar_tensor_tensor(
                                        xs, ul[:], 0.0, xs, ALU.max, ALU.add)
                    nc.sync.dma_start(out=d_out[half], in_=x1[:])
    nc.finalize()
    return nc


def _prep_inputs(inputs):
    bf = ml_dtypes.bfloat16
    A = np.asarray(inputs["A"], np.float32)
    amat = np.ascontiguousarray(A.T).astype(bf)
    halves = {}
    for half in ("bo", "po"):
        pre = "board" if half == "bo" else "po"
        w0 = np.asarray(inputs[f"{pre}_W0"], np.float32)
        d = {f"w0_{half}": np.ascontiguousarray(w0.transpose(1, 0, 2)).astype(bf)}
        if NUM_BLOCKS > 1:
            w = np.asarray(inputs[f"{pre}_W"], np.float32)[:NUM_BLOCKS - 1]
            wp = w.reshape(NUM_BLOCKS - 1, N, 2, 128, EMB).transpose(0, 3, 2, 1, 4)
            d[f"w_{half}"] = np.ascontiguousarray(wp).astype(bf)
        gam = np.asarray(inputs[f"{pre}_gamma"], np.float32)[:NUM_BLOCKS]
        bet = np.asarray(inputs[f"{pre}_beta"], np.float32)[:NUM_BLOCKS]
        g = np.zeros((NP, 2 * NUM_BLOCKS), np.float32)
        g[:N, 0::2] = gam.T
        g[:N, 1::2] = bet.T
        d[f"g_{half}"] = g
        halves[half] = d
    core_ins = []
    for c in range(N_CORES):
        m = {"amat": amat}
        for half in ("bo", "po"):
            m.update(halves[half])
            x = np.asarray(inputs["x_bo" if half == "bo" else "x_po"], np.float32)
            xs = x[c * BL:(c + 1) * BL]
            m[f"x_{half}"] = np.ascontiguousarray(xs.transpose(2, 1, 0)).astype(bf)
        core_ins.append(m)
    return core_ins


def _unpack_outputs(results):
    out = np.zeros((B, N, 2 * EMB), np.float32)
    for c in range(N_CORES):
        for j, half in enumerate(("bo", "po")):
            r = np.asarray(results[c][f"out_{half}"]).astype(np.float32)
            r = r.reshape(128, BL, 2, NP)[:, :, :, :N]
            out[c * BL:(c + 1) * BL, :, j * EMB:(j + 1) * EMB] = \
                r.transpose(1, 3, 2, 0).reshape(BL, N, EMB)
    return out


def kernel(**inputs) -> np.ndarray:
    from concourse.bass_utils import run_bass_kernel_spmd

    if "nc" not in _cache:
        _cache["nc"] = _build()
    core_ins = _prep_inputs(inputs)
    res = run_bass_kernel_spmd(_cache["nc"], core_ins, core_ids=list(range(N_CORES)),
                               trace=bool(int(os.environ.get("KERNEL_TRACE", "0"))))
    _cache["last_result"] = res
    return _unpack_outputs(res.results)

